# revision 9
# baseline (speedup 1.0000x reference)
"""Trainium2 Bass kernel for nn_BatchLsReftIntervention.

Reference semantics (per batch element b):
    z[r]      = dot(base[r, :], w)                       # [S] row matvec
    detect[s] = relu(z[pos[s]])                          # gather of scalars
    topk over detect (K=32, jax tie-break: lowest index wins among equal values)
    non_topk  = detect with the K top entries zeroed
    steer     = mean(topk_vals) * w
    mixed     = base; mixed[pos[s], :] += steer for each s   (duplicates accumulate)
              = base[r, :] + count[r] * mean(topk) * w       (count = histogram of pos)

Sharding: pure data parallel, one batch element per NeuronCore (B=8, 8 cores).

Per-core plan (S=4096, H=2048, P=128 partitions, T=32 row tiles of 128 rows,
row r lives at [partition r//32, free r%32] everywhere -> all [S] arrays are
natural [128, 32] SBUF tiles and row tile t is base[(p*32+t), :]):

  pass 1:  stream base tiles [128, 2048], fused DVE multiply+reduce against a
           broadcast w -> z column per tile.  Meanwhile (PE+DVE, overlapped):
           histogram of pos via one-hot outer-product matmuls:
           count2d[hi, lo] = sum_s onehot64(pos>>6)[s,hi] * onehot64(pos&63)[s,lo].
  middle:  z -> DRAM, indirect-DMA gather z[pos[s]], relu -> detect.
           GPSIMD kth_largest -> tau = exact 32nd largest value of detect.
           j = K - #{detect > tau}; zero entries > tau and the first j entries
           == tau in index order (rank via in-partition scan + triangular
           matmul across partitions) -> non_topk.  mean(topk) =
           (sum of entries > tau + j*tau)/K.
  pass 2:  mixed tile = base tile + (count*mean) x w  (one fused DVE op per
           tile); the last RESIDENT tiles of pass 1 stay in SBUF and skip the
           re-load.
"""

import numpy as np

import concourse.bass as bass
import concourse.mybir as mybir
from concourse import bacc
from concourse.tile import TileContext
from concourse.bass import IndirectOffsetOnAxis
from concourse.bass_utils import run_bass_kernel_spmd

P = 128
S, H, K = 4096, 2048, 32
T = S // P      # 32 row tiles
TS = S // P     # 32 free elems per partition for [S] arrays
LO = 64
HI = S // LO    # 64
RESIDENT = 12   # base tiles kept in SBUF between the passes

F32 = mybir.dt.float32
I32 = mybir.dt.int32
Alu = mybir.AluOpType

N_CORES = 8


def emit(tc, outs, ins, dbg=None):
    nc = tc.nc
    mixed_d, detect_d, nt_d = outs
    base_d, pos_d, w_d = ins

    base_v = base_d.rearrange("(p t) h -> p t h", t=T)
    mixed_v = mixed_d.rearrange("(p t) h -> p t h", t=T)
    pos_v = pos_d.rearrange("(p t) -> p t", t=TS)
    detect_v = detect_d.rearrange("(p t) -> p t", t=TS)
    nt_v = nt_d.rearrange("(p t) -> p t", t=TS)

    with (
        tc.tile_pool(name="const", bufs=1) as const,
        tc.tile_pool(name="hist", bufs=3) as hist,
        tc.tile_pool(name="ld", bufs=4) as ld,
        tc.tile_pool(name="res", bufs=RESIDENT) as res,
        tc.tile_pool(name="sc", bufs=2) as sc,
        tc.tile_pool(name="ps", bufs=1, space="PSUM") as ps,
        tc.tile_pool(name="dr", bufs=1, space="DRAM") as dr,
    ):
        # ---------------- constants / small inputs ----------------
        ones = const.tile([P, P], F32)
        nc.vector.memset(ones[:], 1.0)
        ltd = const.tile([P, P], I32)
        # ltd[p, m] = m - p
        nc.gpsimd.iota(ltd[:], pattern=[[1, P]], base=0, channel_multiplier=-1)
        lt = const.tile([P, P], F32)  # lt[k, m] = 1 if m > k (strict lower tri as lhsT)
        nc.vector.tensor_scalar(
            out=lt[:], in0=ltd[:], scalar1=0, scalar2=None, op0=Alu.is_gt
        )
        zer = const.tile([P, TS], F32)
        nc.vector.memset(zer[:], 0.0)

        pos_sb = const.tile([P, TS], I32)
        nc.sync.dma_start(out=pos_sb[:], in_=pos_v)
        w_row = const.tile([1, H], F32)
        nc.sync.dma_start(out=w_row[:], in_=w_d)
        w_b = const.tile([P, H], F32)
        nc.gpsimd.partition_broadcast(w_b[:], w_row[:])

        # ---------------- histogram of pos via one-hot matmuls ----------------
        i64 = hist.tile([P, TS, LO], I32, tag="hist")
        # i64[p, f, i] = i
        nc.gpsimd.iota(i64[:], pattern=[[0, TS], [1, LO]], base=0, channel_multiplier=0)
        hi_i = const.tile([P, TS], I32)
        nc.vector.tensor_scalar(
            out=hi_i[:], in0=pos_sb[:], scalar1=6, scalar2=None,
            op0=Alu.logical_shift_right,
        )
        lo_i = const.tile([P, TS], I32)
        nc.vector.tensor_scalar(
            out=lo_i[:], in0=pos_sb[:], scalar1=63, scalar2=None, op0=Alu.bitwise_and
        )
        onehot_hi = hist.tile([P, TS, HI], F32, tag="hist")
        nc.vector.tensor_tensor(
            out=onehot_hi[:], in0=i64[:], in1=hi_i[:].to_broadcast([P, TS, HI]),
            op=Alu.is_equal,
        )
        onehot_lo = hist.tile([P, TS, LO], F32, tag="hist")
        nc.vector.tensor_tensor(
            out=onehot_lo[:], in0=i64[:], in1=lo_i[:].to_broadcast([P, TS, LO]),
            op=Alu.is_equal,
        )
        psum_cnt = ps.tile([HI, LO], F32)
        for f in range(TS):
            nc.tensor.matmul(
                out=psum_cnt[:], lhsT=onehot_hi[:, f, :], rhs=onehot_lo[:, f, :],
                start=(f == 0), stop=(f == TS - 1),
            )
        cnt_sq = const.tile([HI, LO], F32)
        nc.vector.tensor_copy(out=cnt_sq[:], in_=psum_cnt[:])
        cnt_dram = dr.tile([S], F32)
        nc.sync.dma_start(
            out=cnt_dram[:].rearrange("(a b) -> a b", b=LO), in_=cnt_sq[:]
        )
        cnt_sb = const.tile([P, TS], F32)
        nc.sync.dma_start(
            out=cnt_sb[:], in_=cnt_dram[:].rearrange("(p t) -> p t", t=TS)
        )

        # ---------------- pass 1: z = base @ w ----------------
        z_sb = const.tile([P, TS], F32)
        base_tiles = {}
        for t in range(T):
            if t >= T - RESIDENT:
                bt = res.tile([P, H], F32, tag="res")
                base_tiles[t] = bt
            else:
                bt = ld.tile([P, H], F32, tag="ld")
            nc.sync.dma_start(out=bt[:], in_=base_v[:, t, :])
            prod = sc.tile([P, H], F32, tag="prod")
            # fused multiply + per-partition sum: z[:, t] = sum_h bt * w
            # (tensor_tensor_reduce crashes the device on this toolchain;
            # scalar_tensor_tensor with accum_out is the working equivalent)
            nc.vector.scalar_tensor_tensor(
                out=prod[:], in0=bt[:], scalar=1.0, in1=w_b[:],
                op0=Alu.mult, op1=Alu.mult, accum_out=z_sb[:, t : t + 1],
            )

        # ---------------- gather + relu -> detect ----------------
        z_dram = dr.tile([S], F32)
        nc.sync.dma_start(
            out=z_dram[:].rearrange("(p t) -> p t", t=TS), in_=z_sb[:]
        )
        # HW honors one offset per partition line, so gather column-by-column
        # ([128,1] offsets -> 128 single-element lines per call).
        gath = const.tile([P, TS], F32)
        for f in range(TS):
            nc.gpsimd.indirect_dma_start(
                out=gath[:, f : f + 1], out_offset=None,
                in_=z_dram[:].rearrange("(s o) -> s o", o=1),
                in_offset=IndirectOffsetOnAxis(ap=pos_sb[:, f : f + 1], axis=0),
            )
        detect = const.tile([P, TS], F32)
        nc.vector.tensor_scalar(
            out=detect[:], in0=gath[:], scalar1=0.0, scalar2=None, op0=Alu.max
        )
        nc.sync.dma_start(out=detect_v, in_=detect[:])

        # ---------------- tau = exact K-th largest of detect ----------------
        kth = const.tile([1, 2], F32)
        q = 1.0 - (K - 1.5) / (S - 1)  # k_adj = K-2  ->  out[0,1] = desc[K-1]
        nc.gpsimd.kth_largest(kth[:], detect[:], n_per_lane=TS, k=K, quantile=q)
        taub = const.tile([P, 2], F32)
        nc.gpsimd.partition_broadcast(taub[:], kth[:])
        tau = taub[:, 1:2]

        # ---------------- masks, tie-break, mean(topk) ----------------
        cs2 = const.tile([P, 2], F32)
        gt01 = const.tile([P, TS], F32)
        nc.vector.tensor_scalar(
            out=gt01[:], in0=detect[:], scalar1=tau, scalar2=None, op0=Alu.is_gt,
            op1=Alu.add, accum_out=cs2[:, 0:1],
        )
        selb = const.tile([P, TS], F32)
        nc.vector.scalar_tensor_tensor(
            out=selb[:], in0=detect[:], scalar=tau, in1=detect[:],
            op0=Alu.is_gt, op1=Alu.mult, accum_out=cs2[:, 1:2],
        )
        eq01 = const.tile([P, TS], F32)
        eqc = const.tile([P, 1], F32)
        nc.vector.tensor_scalar(
            out=eq01[:], in0=detect[:], scalar1=tau, scalar2=None, op0=Alu.is_equal,
            op1=Alu.add, accum_out=eqc[:],
        )
        bro = ps.tile([P, 2], F32)  # [:,0] = total #gt, [:,1] = total sum(gt*detect)
        nc.tensor.matmul(out=bro[:], lhsT=ones[:], rhs=cs2[:], start=True, stop=True)
        off = ps.tile([P, 1], F32)  # exclusive per-partition prefix of eq counts
        nc.tensor.matmul(out=off[:], lhsT=lt[:], rhs=eqc[:], start=True, stop=True)

        pcum = const.tile([P, TS], F32)
        nc.vector.tensor_tensor_scan(
            out=pcum[:], data0=eq01[:], data1=zer[:], initial=0.0,
            op0=Alu.add, op1=Alu.add,
        )
        rank = const.tile([P, TS], F32)
        nc.vector.tensor_scalar(
            out=rank[:], in0=pcum[:], scalar1=off[:], scalar2=None, op0=Alu.add
        )
        jb = const.tile([P, 1], F32)  # j = K - #gt
        nc.vector.tensor_scalar(
            out=jb[:], in0=bro[:, 0:1], scalar1=-1.0, scalar2=float(K),
            op0=Alu.mult, op1=Alu.add,
        )
        lem = const.tile([P, TS], F32)
        nc.vector.tensor_scalar(
            out=lem[:], in0=rank[:], scalar1=jb[:], scalar2=None, op0=Alu.is_le
        )
        zap = const.tile([P, TS], F32)
        nc.vector.tensor_tensor(out=zap[:], in0=lem[:], in1=eq01[:], op=Alu.mult)
        nc.vector.tensor_tensor(out=zap[:], in0=zap[:], in1=gt01[:], op=Alu.add)
        t4 = const.tile([P, TS], F32)
        nc.vector.tensor_tensor(out=t4[:], in0=detect[:], in1=zap[:], op=Alu.mult)
        nt = const.tile([P, TS], F32)
        nc.vector.tensor_tensor(out=nt[:], in0=detect[:], in1=t4[:], op=Alu.subtract)
        nc.sync.dma_start(out=nt_v, in_=nt[:])

        if dbg is not None:
            nc.sync.dma_start(
                out=dbg["z"].rearrange("(p t) -> p t", t=TS), in_=z_sb[:]
            )
            nc.sync.dma_start(
                out=dbg["gath"].rearrange("(p t) -> p t", t=TS), in_=gath[:]
            )
            nc.sync.dma_start(
                out=dbg["cnt"].rearrange("(p t) -> p t", t=TS), in_=cnt_sb[:]
            )
            nc.sync.dma_start(out=dbg["kth"], in_=kth[:])
            nc.sync.dma_start(
                out=dbg["rank"].rearrange("(p t) -> p t", t=TS), in_=rank[:]
            )

        # mean(topk) = (sum(detect>tau) + j*tau) / K, then fold in count
        t2 = const.tile([P, 1], F32)
        nc.vector.tensor_tensor(out=t2[:], in0=jb[:], in1=tau, op=Alu.mult)
        nc.vector.tensor_tensor(out=t2[:], in0=t2[:], in1=bro[:, 1:2], op=Alu.add)
        meanms = const.tile([P, 1], F32)
        nc.vector.tensor_scalar(
            out=meanms[:], in0=t2[:], scalar1=1.0 / K, scalar2=None, op0=Alu.mult
        )
        scol = const.tile([P, TS], F32)  # scol[p, t] = count * mean
        nc.vector.tensor_scalar(
            out=scol[:], in0=cnt_sb[:], scalar1=meanms[:], scalar2=None, op0=Alu.mult
        )

        # ---------------- pass 2: mixed = base + scol x w ----------------
        for t in range(T - 1, -1, -1):  # resident (last-loaded) tiles first
            if t >= T - RESIDENT:
                bt = base_tiles[t]
            else:
                bt = ld.tile([P, H], F32, tag="ld")
                nc.sync.dma_start(out=bt[:], in_=base_v[:, t, :])
            nc.vector.scalar_tensor_tensor(
                out=bt[:], in0=w_b[:], scalar=scol[:, t : t + 1], in1=bt[:],
                op0=Alu.mult, op1=Alu.add,
            )
            nc.sync.dma_start(out=mixed_v[:, t, :], in_=bt[:])


_NC_CACHE = None


def _build():
    global _NC_CACHE
    if _NC_CACHE is None:
        nc = bacc.Bacc("TRN2", target_bir_lowering=False, debug=False)
        base_t = nc.dram_tensor("base", [S, H], F32, kind="ExternalInput")
        pos_t = nc.dram_tensor("pos", [S], I32, kind="ExternalInput")
        w_t = nc.dram_tensor("w", [1, H], F32, kind="ExternalInput")
        mix_t = nc.dram_tensor("mixed", [S, H], F32, kind="ExternalOutput")
        det_t = nc.dram_tensor("detect", [S], F32, kind="ExternalOutput")
        nt_t = nc.dram_tensor("non_topk", [S], F32, kind="ExternalOutput")
        with TileContext(nc) as tc:
            emit(
                tc,
                (mix_t[:], det_t[:], nt_t[:]),
                (base_t[:], pos_t[:], w_t[:]),
            )
        nc.compile()
        _NC_CACHE = nc
    return _NC_CACHE


def run(base, intervention_positions, batch_weights, **spmd_kwargs):
    """Run the kernel; returns (outputs_tuple, BassKernelResults)."""
    nc = _build()
    base = np.asarray(base, dtype=np.float32)
    pos = np.asarray(intervention_positions, dtype=np.int32)
    w = np.asarray(batch_weights, dtype=np.float32)
    in_maps = [
        {
            "base": np.ascontiguousarray(base[i]),
            "pos": np.ascontiguousarray(pos[i]),
            "w": np.ascontiguousarray(w[i]),
        }
        for i in range(N_CORES)
    ]
    results = run_bass_kernel_spmd(nc, in_maps, list(range(N_CORES)), **spmd_kwargs)
    outs = results.results
    mixed = np.stack([outs[i]["mixed"] for i in range(N_CORES)])
    detect = np.stack([outs[i]["detect"] for i in range(N_CORES)])
    non_topk = np.stack([outs[i]["non_topk"] for i in range(N_CORES)])
    return (mixed, detect, non_topk), results


def kernel(base, intervention_positions, batch_weights):
    out, _ = run(base, intervention_positions, batch_weights)
    return out


# revision 25
# speedup vs baseline: 5.6054x; 5.6054x over previous
"""Trainium2 Bass kernel for nn_BatchLsReftIntervention.

Reference semantics (per batch element b):
    z[r]      = dot(base[r, :], w)                       # [S] row matvec
    detect[s] = relu(z[pos[s]])                          # gather of scalars
    topk over detect (K=32, jax tie-break: lowest index wins among equal values)
    non_topk  = detect with the K top entries zeroed
    steer     = mean(topk_vals) * w
    mixed     = base; mixed[pos[s], :] += steer for each s   (duplicates accumulate)
              = base[r, :] + count[r] * mean(topk) * w       (count = histogram of pos)

Sharding: pure data parallel, one batch element per NeuronCore (B=8, 8 cores).

Per-core plan (S=4096, H=2048, P=128 partitions, T=32 row tiles of 128 rows,
row r lives at [partition r//32, free r%32] everywhere -> all [S] arrays are
natural [128, 32] SBUF tiles and row tile t is base[(p*32+t), :]):

  pass 1:  stream base tiles [128, 2048], fused DVE multiply+reduce against a
           broadcast w -> z column per tile.  Meanwhile (PE+DVE, overlapped):
           histogram of pos via one-hot outer-product matmuls:
           count2d[hi, lo] = sum_s onehot64(pos>>6)[s,hi] * onehot64(pos&63)[s,lo].
  middle:  z -> DRAM, indirect-DMA gather z[pos[s]], relu -> detect.
           GPSIMD kth_largest -> tau = exact 32nd largest value of detect.
           j = K - #{detect > tau}; zero entries > tau and the first j entries
           == tau in index order (rank via in-partition scan + triangular
           matmul across partitions) -> non_topk.  mean(topk) =
           (sum of entries > tau + j*tau)/K.
  pass 2:  mixed tile = base tile + (count*mean) x w  (one fused DVE op per
           tile); the last RESIDENT tiles of pass 1 stay in SBUF and skip the
           re-load.
"""

import numpy as np

import concourse.bass as bass
import concourse.mybir as mybir
from concourse import bacc
from concourse.tile import TileContext
from concourse.bass import IndirectOffsetOnAxis
from concourse.bass_utils import run_bass_kernel_spmd

P = 128
S, H, K = 4096, 2048, 32
T = S // P      # 32 row tiles
TS = S // P     # 32 free elems per partition for [S] arrays
LO = 64
HI = S // LO    # 64
RESIDENT = 13   # base tiles kept in SBUF between the passes

F32 = mybir.dt.float32
I32 = mybir.dt.int32
I16 = mybir.dt.int16
Alu = mybir.AluOpType

N_CORES = 8


def emit(tc, outs, ins, dbg=None, variant="full"):
    nc = tc.nc
    mixed_d, detect_d, nt_d = outs
    base_d, pos_d, w_d = ins

    base_v = base_d.rearrange("(p t) h -> p t h", t=T)
    mixed_v = mixed_d.rearrange("(p t) h -> p t h", t=T)
    pos_v = pos_d.rearrange("(p t) -> p t", t=TS)
    detect_v = detect_d.rearrange("(p t) -> p t", t=TS)
    nt_v = nt_d.rearrange("(p t) -> p t", t=TS)

    with (
        tc.tile_pool(name="const", bufs=1) as const,
        tc.tile_pool(name="hist", bufs=3) as hist,
        tc.tile_pool(name="ld", bufs=3) as ld,
        tc.tile_pool(name="res", bufs=RESIDENT) as res,
        tc.tile_pool(name="sc", bufs=2) as sc,
        tc.tile_pool(name="zt", bufs=1) as zt,
        tc.tile_pool(name="ps", bufs=1, space="PSUM") as ps,
        tc.tile_pool(name="dr", bufs=1, space="DRAM") as dr,
    ):
        # ---------------- constants / small inputs ----------------
        ones = const.tile([P, P], F32)
        nc.vector.memset(ones[:], 1.0)
        ltd = const.tile([P, P], I32)
        # ltd[p, m] = m - p
        nc.gpsimd.iota(ltd[:], pattern=[[1, P]], base=0, channel_multiplier=-1)
        lt = const.tile([P, P], F32)  # lt[k, m] = 1 if m > k (strict lower tri as lhsT)
        nc.vector.tensor_scalar(
            out=lt[:], in0=ltd[:], scalar1=0, scalar2=None, op0=Alu.is_gt
        )
        zer = const.tile([P, TS], F32)
        nc.vector.memset(zer[:], 0.0)

        pos_sb = const.tile([P, TS], I32)
        nc.sync.dma_start(out=pos_sb[:], in_=pos_v)
        w_row = const.tile([1, H], F32)
        nc.sync.dma_start(out=w_row[:], in_=w_d)
        w_b = const.tile([P, H], F32)
        nc.gpsimd.partition_broadcast(w_b[:], w_row[:])
        ztab = zt.tile([P, S], F32)  # gather table; zeroed here, filled later
        nc.vector.memset(ztab[:], 0.0)

        # ---------------- histogram of pos via one-hot matmuls ----------------
        i64 = hist.tile([P, TS, LO], I32, tag="hist")
        # i64[p, f, i] = i
        nc.gpsimd.iota(i64[:], pattern=[[0, TS], [1, LO]], base=0, channel_multiplier=0)
        hi_i = const.tile([P, TS], I32)
        nc.vector.tensor_scalar(
            out=hi_i[:], in0=pos_sb[:], scalar1=6, scalar2=None,
            op0=Alu.logical_shift_right,
        )
        lo_i = const.tile([P, TS], I32)
        nc.vector.tensor_scalar(
            out=lo_i[:], in0=pos_sb[:], scalar1=63, scalar2=None, op0=Alu.bitwise_and
        )
        onehot_hi = hist.tile([P, TS, HI], F32, tag="hist")
        nc.vector.tensor_tensor(
            out=onehot_hi[:], in0=i64[:], in1=hi_i[:].to_broadcast([P, TS, HI]),
            op=Alu.is_equal,
        )
        onehot_lo = hist.tile([P, TS, LO], F32, tag="hist")
        nc.vector.tensor_tensor(
            out=onehot_lo[:], in0=i64[:], in1=lo_i[:].to_broadcast([P, TS, LO]),
            op=Alu.is_equal,
        )
        psum_cnt = ps.tile([HI, LO], F32)
        for f in range(TS):
            nc.tensor.matmul(
                out=psum_cnt[:], lhsT=onehot_hi[:, f, :], rhs=onehot_lo[:, f, :],
                start=(f == 0), stop=(f == TS - 1),
            )
        cnt_sq = const.tile([HI, LO], F32)
        nc.vector.tensor_copy(out=cnt_sq[:], in_=psum_cnt[:])
        cnt_dram = dr.tile([S], F32)
        nc.sync.dma_start(
            out=cnt_dram[:].rearrange("(a b) -> a b", b=LO), in_=cnt_sq[:]
        )
        cnt_sb = const.tile([P, TS], F32)
        nc.sync.dma_start(
            out=cnt_sb[:], in_=cnt_dram[:].rearrange("(p t) -> p t", t=TS)
        )

        # ---------------- pass 1: z = base @ w ----------------
        # The last RESIDENT tiles stay in SBUF for pass 2; three more reuse
        # the histogram pool slots (dead after the one-hot matmuls).
        z_sb = const.tile([P, TS], F32)
        base_tiles = {}
        n_res = RESIDENT + 3
        for t in range(T):
            if t >= T - RESIDENT:
                bt = res.tile([P, H], F32, tag="res")
                base_tiles[t] = bt
            elif t >= T - n_res:
                bt = hist.tile([P, H], F32, tag="hist")
                base_tiles[t] = bt
            else:
                bt = ld.tile([P, H], F32, tag="ld")
            nc.sync.dma_start(out=bt[:], in_=base_v[:, t, :])
            prod = sc.tile([P, H], mybir.dt.bfloat16, tag="prod")
            # fused multiply + per-partition sum: z[:, t] = sum_h bt * w
            # (tensor_tensor_reduce crashes the device on this toolchain;
            # scalar_tensor_tensor with accum_out is the working equivalent)
            nc.vector.scalar_tensor_tensor(
                out=prod[:], in0=bt[:], scalar=1.0, in1=w_b[:],
                op0=Alu.mult, op1=Alu.mult, accum_out=z_sb[:, t : t + 1],
            )

        # ---------------- gather + relu -> detect ----------------
        z_dram = dr.tile([S], F32)
        nc.sync.dma_start(
            out=z_dram[:].rearrange("(p t) -> p t", t=TS), in_=z_sb[:]
        )
        gath = const.tile([P, TS], F32)
        if variant == "nogather":
            nc.vector.tensor_copy(out=gath[:], in_=z_sb[:])
        elif variant == "vcols":
            # indirect DMA honors one offset per partition line -> one call
            # per column ([128,1] offsets, 128 single-element lines each)
            for f in range(TS):
                nc.gpsimd.indirect_dma_start(
                    out=gath[:, f : f + 1], out_offset=None,
                    in_=z_dram[:].rearrange("(s o) -> s o", o=1),
                    in_offset=IndirectOffsetOnAxis(ap=pos_sb[:, f : f + 1], axis=0),
                )
        else:
            # GPSIMD ap_gather against a partition-replicated z table.
            # idx layout per 16-partition core group c:
            #   idx[16c + q, jj] = pos[512c + 16jj + q]  (int16)
            # gathered row 16c then holds detect for s in [512c, 512c+512).
            idx32 = const.tile([P, TS], I32)
            iv = pos_d.rearrange("(c jj q) -> c q jj", c=8, jj=TS)
            for c in range(8):
                nc.sync.dma_start(
                    out=idx32[16 * c : 16 * c + 16, :], in_=iv[c, :, :]
                )
            idx16 = const.tile([P, TS], I16)
            nc.vector.tensor_copy(out=idx16[:], in_=idx32[:])

            # the gather only uses row 16c of each core group's output, so
            # the table needs valid data only on partitions {16c}; fill those
            # by DMA instead of a full 2 MiB partition_broadcast (ztab_bg was
            # zeroed during setup so the unused rows read as initialized)
            zrow = z_dram[:].rearrange("(o s) -> o s", o=1)
            for c in range(8):
                nc.sync.dma_start(out=ztab[16 * c : 16 * c + 1, :], in_=zrow)
            gbig = const.tile([P, S // 8], F32)
            nc.gpsimd.ap_gather(gbig[:], ztab[:], idx16[:], channels=P,
                                num_elems=S, d=1, num_idxs=S // 8)
            gv = gbig[:].rearrange("p (q f) -> p q f", q=16)
            nc.sync.dma_start(out=gath[:], in_=gv[::16, :, :])
        detect = const.tile([P, TS], F32)
        nc.vector.tensor_scalar(
            out=detect[:], in0=gath[:], scalar1=0.0, scalar2=None, op0=Alu.max
        )
        nc.sync.dma_start(out=detect_v, in_=detect[:])

        if variant == "nomid":
            nc.sync.dma_start(out=nt_v, in_=detect[:])
            scol = const.tile([P, TS], F32)
            nc.vector.tensor_copy(out=scol[:], in_=cnt_sb[:])
            for t in range(T - 1, -1, -1):
                if t in base_tiles:
                    bt = base_tiles[t]
                else:
                    bt = ld.tile([P, H], F32, tag="ld")
                    nc.sync.dma_start(out=bt[:], in_=base_v[:, t, :])
                nc.vector.scalar_tensor_tensor(
                    out=bt[:], in0=w_b[:], scalar=scol[:, t : t + 1], in1=bt[:],
                    op0=Alu.mult, op1=Alu.add,
                )
                nc.sync.dma_start(out=mixed_v[:, t, :], in_=bt[:])
            return

        # ---------------- tau = exact K-th largest of detect ----------------
        kth = const.tile([1, 2], F32)
        q = 1.0 - (K - 1.5) / (S - 1)  # k_adj = K-2  ->  out[0,1] = desc[K-1]
        nc.gpsimd.kth_largest(kth[:], detect[:], n_per_lane=TS, k=K, quantile=q)
        taub = const.tile([P, 2], F32)
        nc.gpsimd.partition_broadcast(taub[:], kth[:])
        tau = taub[:, 1:2]

        # ---------------- masks, tie-break, mean(topk) ----------------
        cs2 = const.tile([P, 2], F32)
        gt01 = const.tile([P, TS], F32)
        nc.vector.tensor_scalar(
            out=gt01[:], in0=detect[:], scalar1=tau, scalar2=None, op0=Alu.is_gt,
            op1=Alu.add, accum_out=cs2[:, 0:1],
        )
        selb = const.tile([P, TS], F32)
        nc.vector.scalar_tensor_tensor(
            out=selb[:], in0=detect[:], scalar=tau, in1=detect[:],
            op0=Alu.is_gt, op1=Alu.mult, accum_out=cs2[:, 1:2],
        )
        eq01 = const.tile([P, TS], F32)
        eqc = const.tile([P, 1], F32)
        nc.vector.tensor_scalar(
            out=eq01[:], in0=detect[:], scalar1=tau, scalar2=None, op0=Alu.is_equal,
            op1=Alu.add, accum_out=eqc[:],
        )
        bro = ps.tile([P, 2], F32)  # [:,0] = total #gt, [:,1] = total sum(gt*detect)
        nc.tensor.matmul(out=bro[:], lhsT=ones[:], rhs=cs2[:], start=True, stop=True)
        off = ps.tile([P, 1], F32)  # exclusive per-partition prefix of eq counts
        nc.tensor.matmul(out=off[:], lhsT=lt[:], rhs=eqc[:], start=True, stop=True)

        pcum = const.tile([P, TS], F32)
        nc.vector.tensor_tensor_scan(
            out=pcum[:], data0=eq01[:], data1=zer[:], initial=0.0,
            op0=Alu.add, op1=Alu.add,
        )
        rank = const.tile([P, TS], F32)
        nc.vector.tensor_scalar(
            out=rank[:], in0=pcum[:], scalar1=off[:], scalar2=None, op0=Alu.add
        )
        jb = const.tile([P, 1], F32)  # j = K - #gt
        nc.vector.tensor_scalar(
            out=jb[:], in0=bro[:, 0:1], scalar1=-1.0, scalar2=float(K),
            op0=Alu.mult, op1=Alu.add,
        )
        lem = const.tile([P, TS], F32)
        nc.vector.tensor_scalar(
            out=lem[:], in0=rank[:], scalar1=jb[:], scalar2=None, op0=Alu.is_le
        )
        zap = const.tile([P, TS], F32)
        nc.vector.tensor_tensor(out=zap[:], in0=lem[:], in1=eq01[:], op=Alu.mult)
        nc.vector.tensor_tensor(out=zap[:], in0=zap[:], in1=gt01[:], op=Alu.add)
        t4 = const.tile([P, TS], F32)
        nc.vector.tensor_tensor(out=t4[:], in0=detect[:], in1=zap[:], op=Alu.mult)
        nt = const.tile([P, TS], F32)
        nc.vector.tensor_tensor(out=nt[:], in0=detect[:], in1=t4[:], op=Alu.subtract)
        nc.sync.dma_start(out=nt_v, in_=nt[:])

        if dbg is not None:
            nc.sync.dma_start(
                out=dbg["z"].rearrange("(p t) -> p t", t=TS), in_=z_sb[:]
            )
            nc.sync.dma_start(
                out=dbg["gath"].rearrange("(p t) -> p t", t=TS), in_=gath[:]
            )
            nc.sync.dma_start(
                out=dbg["cnt"].rearrange("(p t) -> p t", t=TS), in_=cnt_sb[:]
            )
            nc.sync.dma_start(out=dbg["kth"], in_=kth[:])
            nc.sync.dma_start(
                out=dbg["rank"].rearrange("(p t) -> p t", t=TS), in_=rank[:]
            )

        # mean(topk) = (sum(detect>tau) + j*tau) / K, then fold in count
        t2 = const.tile([P, 1], F32)
        nc.vector.tensor_tensor(out=t2[:], in0=jb[:], in1=tau, op=Alu.mult)
        nc.vector.tensor_tensor(out=t2[:], in0=t2[:], in1=bro[:, 1:2], op=Alu.add)
        meanms = const.tile([P, 1], F32)
        nc.vector.tensor_scalar(
            out=meanms[:], in0=t2[:], scalar1=1.0 / K, scalar2=None, op0=Alu.mult
        )
        scol = const.tile([P, TS], F32)  # scol[p, t] = count * mean
        nc.vector.tensor_scalar(
            out=scol[:], in0=cnt_sb[:], scalar1=meanms[:], scalar2=None, op0=Alu.mult
        )

        # ---------------- pass 2: mixed = base + scol x w ----------------
        for t in range(T):  # non-resident first: loads prefetch in the middle
            if t in base_tiles:
                bt = base_tiles[t]
            else:
                bt = ld.tile([P, H], F32, tag="ld")
                nc.sync.dma_start(out=bt[:], in_=base_v[:, t, :])
            nc.vector.scalar_tensor_tensor(
                out=bt[:], in0=w_b[:], scalar=scol[:, t : t + 1], in1=bt[:],
                op0=Alu.mult, op1=Alu.add,
            )
            nc.sync.dma_start(out=mixed_v[:, t, :], in_=bt[:])


_NC_CACHE = None


def _build():
    global _NC_CACHE
    if _NC_CACHE is None:
        nc = bacc.Bacc("TRN2", target_bir_lowering=False, debug=False)
        base_t = nc.dram_tensor("base", [S, H], F32, kind="ExternalInput")
        pos_t = nc.dram_tensor("pos", [S], I32, kind="ExternalInput")
        w_t = nc.dram_tensor("w", [1, H], F32, kind="ExternalInput")
        mix_t = nc.dram_tensor("mixed", [S, H], F32, kind="ExternalOutput")
        det_t = nc.dram_tensor("detect", [S], F32, kind="ExternalOutput")
        nt_t = nc.dram_tensor("non_topk", [S], F32, kind="ExternalOutput")
        with TileContext(nc) as tc:
            emit(
                tc,
                (mix_t[:], det_t[:], nt_t[:]),
                (base_t[:], pos_t[:], w_t[:]),
            )
        nc.compile()
        _NC_CACHE = nc
    return _NC_CACHE


def run(base, intervention_positions, batch_weights, **spmd_kwargs):
    """Run the kernel; returns (outputs_tuple, BassKernelResults)."""
    nc = _build()
    base = np.asarray(base, dtype=np.float32)
    pos = np.asarray(intervention_positions, dtype=np.int32)
    w = np.asarray(batch_weights, dtype=np.float32)
    in_maps = [
        {
            "base": np.ascontiguousarray(base[i]),
            "pos": np.ascontiguousarray(pos[i]),
            "w": np.ascontiguousarray(w[i]),
        }
        for i in range(N_CORES)
    ]
    results = run_bass_kernel_spmd(nc, in_maps, list(range(N_CORES)), **spmd_kwargs)
    outs = results.results
    mixed = np.stack([outs[i]["mixed"] for i in range(N_CORES)])
    detect = np.stack([outs[i]["detect"] for i in range(N_CORES)])
    non_topk = np.stack([outs[i]["non_topk"] for i in range(N_CORES)])
    return (mixed, detect, non_topk), results


def kernel(base, intervention_positions, batch_weights):
    out, _ = run(base, intervention_positions, batch_weights)
    return out


# revision 27
# speedup vs baseline: 11.0448x; 1.9704x over previous
"""Trainium2 Bass kernel for nn_BatchLsReftIntervention.

Reference semantics (per batch element b):
    z[r]      = dot(base[r, :], w)                       # [S] row matvec
    detect[s] = relu(z[pos[s]])                          # gather of scalars
    topk over detect (K=32, jax tie-break: lowest index wins among equal values)
    non_topk  = detect with the K top entries zeroed
    steer     = mean(topk_vals) * w
    mixed     = base; mixed[pos[s], :] += steer for each s   (duplicates accumulate)
              = base[r, :] + count[r] * mean(topk) * w       (count = histogram of pos)

Sharding: pure data parallel, one batch element per NeuronCore (B=8, 8 cores).

Per-core plan (S=4096, H=2048, P=128 partitions, T=32 row tiles of 128 rows,
row r lives at [partition r//32, free r%32] everywhere -> all [S] arrays are
natural [128, 32] SBUF tiles and row tile t is base[(p*32+t), :]):

  pass 1:  stream base tiles [128, 2048], fused DVE multiply+reduce against a
           broadcast w -> z column per tile.  Meanwhile (PE+DVE, overlapped):
           histogram of pos via one-hot outer-product matmuls:
           count2d[hi, lo] = sum_s onehot64(pos>>6)[s,hi] * onehot64(pos&63)[s,lo].
  middle:  z -> DRAM -> per-core-group rows of an SBUF table; GPSIMD
           ap_gather z[pos[s]], relu -> detect.
           GPSIMD kth_largest -> tau = exact 32nd largest value of detect.
           j = K - #{detect > tau}; zero entries > tau and the first j entries
           == tau in index order (rank via in-partition scan + triangular
           matmul across partitions) -> non_topk.  mean(topk) =
           (sum of entries > tau + j*tau)/K.
  pass 2:  mixed tile = base tile + (count*mean) x w  (one fused DVE op per
           tile); the last RESIDENT tiles of pass 1 stay in SBUF and skip the
           re-load.
"""

import numpy as np

import concourse.bass as bass
import concourse.mybir as mybir
from concourse import bacc
from concourse.tile import TileContext
from concourse.bass import IndirectOffsetOnAxis
from concourse.bass_utils import run_bass_kernel_spmd

P = 128
S, H, K = 4096, 2048, 32
T = S // P      # 32 row tiles
TS = S // P     # 32 free elems per partition for [S] arrays
LO = 64
HI = S // LO    # 64
RESIDENT = 13   # base tiles kept in SBUF between the passes

F32 = mybir.dt.float32
I32 = mybir.dt.int32
I16 = mybir.dt.int16
Alu = mybir.AluOpType

N_CORES = 8


def emit(tc, outs, ins, dbg=None, variant="full"):
    nc = tc.nc
    mixed_d, detect_d, nt_d = outs
    base_d, pos_d, w_d = ins

    base_v = base_d.rearrange("(p t) h -> p t h", t=T)
    mixed_v = mixed_d.rearrange("(p t) h -> p t h", t=T)
    pos_v = pos_d.rearrange("(p t) -> p t", t=TS)
    detect_v = detect_d.rearrange("(p t) -> p t", t=TS)
    nt_v = nt_d.rearrange("(p t) -> p t", t=TS)

    with (
        tc.tile_pool(name="const", bufs=1) as const,
        tc.tile_pool(name="hist", bufs=3) as hist,
        tc.tile_pool(name="ld", bufs=3) as ld,
        tc.tile_pool(name="res", bufs=RESIDENT) as res,
        tc.tile_pool(name="sc", bufs=2) as sc,
        tc.tile_pool(name="zt", bufs=1) as zt,
        tc.tile_pool(name="ps", bufs=1, space="PSUM") as ps,
        tc.tile_pool(name="dr", bufs=1, space="DRAM") as dr,
    ):
        # ---------------- constants / small inputs ----------------
        ones = const.tile([P, P], F32)
        nc.vector.memset(ones[:], 1.0)
        ltd = const.tile([P, P], I32)
        # ltd[p, m] = m - p
        nc.gpsimd.iota(ltd[:], pattern=[[1, P]], base=0, channel_multiplier=-1)
        lt = const.tile([P, P], F32)  # lt[k, m] = 1 if m > k (strict lower tri as lhsT)
        nc.vector.tensor_scalar(
            out=lt[:], in0=ltd[:], scalar1=0, scalar2=None, op0=Alu.is_gt
        )
        zer = const.tile([P, TS], F32)
        nc.vector.memset(zer[:], 0.0)

        pos_sb = const.tile([P, TS], I32)
        nc.sync.dma_start(out=pos_sb[:], in_=pos_v)
        w_row = const.tile([1, H], F32)
        nc.sync.dma_start(out=w_row[:], in_=w_d)
        w_b = const.tile([P, H], F32)
        nc.gpsimd.partition_broadcast(w_b[:], w_row[:])
        ztab = zt.tile([P, S], F32)  # gather table; zeroed here, filled later
        nc.vector.memset(ztab[:], 0.0)

        # ---------------- histogram of pos via one-hot matmuls ----------------
        i64 = hist.tile([P, TS, LO], I32, tag="hist")
        # i64[p, f, i] = i
        nc.gpsimd.iota(i64[:], pattern=[[0, TS], [1, LO]], base=0, channel_multiplier=0)
        hi_i = const.tile([P, TS], I32)
        nc.vector.tensor_scalar(
            out=hi_i[:], in0=pos_sb[:], scalar1=6, scalar2=None,
            op0=Alu.logical_shift_right,
        )
        lo_i = const.tile([P, TS], I32)
        nc.vector.tensor_scalar(
            out=lo_i[:], in0=pos_sb[:], scalar1=63, scalar2=None, op0=Alu.bitwise_and
        )
        onehot_hi = hist.tile([P, TS, HI], F32, tag="hist")
        nc.vector.tensor_tensor(
            out=onehot_hi[:], in0=i64[:], in1=hi_i[:].to_broadcast([P, TS, HI]),
            op=Alu.is_equal,
        )
        onehot_lo = hist.tile([P, TS, LO], F32, tag="hist")
        nc.vector.tensor_tensor(
            out=onehot_lo[:], in0=i64[:], in1=lo_i[:].to_broadcast([P, TS, LO]),
            op=Alu.is_equal,
        )
        psum_cnt = ps.tile([HI, LO], F32)
        for f in range(TS):
            nc.tensor.matmul(
                out=psum_cnt[:], lhsT=onehot_hi[:, f, :], rhs=onehot_lo[:, f, :],
                start=(f == 0), stop=(f == TS - 1),
            )
        cnt_sq = const.tile([HI, LO], F32)
        nc.vector.tensor_copy(out=cnt_sq[:], in_=psum_cnt[:])
        cnt_dram = dr.tile([S], F32)
        nc.sync.dma_start(
            out=cnt_dram[:].rearrange("(a b) -> a b", b=LO), in_=cnt_sq[:]
        )
        cnt_sb = const.tile([P, TS], F32)
        nc.sync.dma_start(
            out=cnt_sb[:], in_=cnt_dram[:].rearrange("(p t) -> p t", t=TS)
        )

        # ---------------- pass 1: z = base @ w ----------------
        # The last RESIDENT tiles stay in SBUF for pass 2; three more reuse
        # the histogram pool slots (dead after the one-hot matmuls).
        z_sb = const.tile([P, TS], F32)
        base_tiles = {}
        n_res = RESIDENT + 3
        for t in range(T):
            if t >= T - RESIDENT:
                bt = res.tile([P, H], F32, tag="res")
                base_tiles[t] = bt
            elif t >= T - n_res:
                bt = hist.tile([P, H], F32, tag="hist")
                base_tiles[t] = bt
            else:
                bt = ld.tile([P, H], F32, tag="ld")
            nc.sync.dma_start(out=bt[:], in_=base_v[:, t, :])
            prod = sc.tile([P, H], mybir.dt.bfloat16, tag="prod")
            # fused multiply + per-partition sum: z[:, t] = sum_h bt * w
            # (tensor_tensor_reduce crashes the device on this toolchain;
            # scalar_tensor_tensor with accum_out is the working equivalent)
            nc.vector.scalar_tensor_tensor(
                out=prod[:], in0=bt[:], scalar=1.0, in1=w_b[:],
                op0=Alu.mult, op1=Alu.mult, accum_out=z_sb[:, t : t + 1],
            )

        # ---------------- gather + relu -> detect ----------------
        z_dram = dr.tile([S], F32)
        nc.sync.dma_start(
            out=z_dram[:].rearrange("(p t) -> p t", t=TS), in_=z_sb[:]
        )
        gath = const.tile([P, TS], F32)
        if variant == "nogather":
            nc.vector.tensor_copy(out=gath[:], in_=z_sb[:])
        elif variant == "vcols":
            # indirect DMA honors one offset per partition line -> one call
            # per column ([128,1] offsets, 128 single-element lines each)
            for f in range(TS):
                nc.gpsimd.indirect_dma_start(
                    out=gath[:, f : f + 1], out_offset=None,
                    in_=z_dram[:].rearrange("(s o) -> s o", o=1),
                    in_offset=IndirectOffsetOnAxis(ap=pos_sb[:, f : f + 1], axis=0),
                )
        else:
            # GPSIMD ap_gather against a partition-replicated z table.
            # idx layout per 16-partition core group c:
            #   idx[16c + q, jj] = pos[512c + 16jj + q]  (int16)
            # gathered row 16c then holds detect for s in [512c, 512c+512).
            idx32 = const.tile([P, TS], I32)
            iv = pos_d.rearrange("(c jj q) -> c q jj", c=8, jj=TS)
            for c in range(8):
                nc.sync.dma_start(
                    out=idx32[16 * c : 16 * c + 16, :], in_=iv[c, :, :]
                )
            idx16 = const.tile([P, TS], I16)
            nc.vector.tensor_copy(out=idx16[:], in_=idx32[:])

            # the gather only uses row 16c of each core group's output, so
            # the table needs valid data only on partitions {16c}; fill those
            # by DMA instead of a full 2 MiB partition_broadcast (ztab was
            # zeroed during setup so the unused rows read as initialized)
            zrow = z_dram[:].rearrange("(o s) -> o s", o=1)
            for c in range(8):
                nc.sync.dma_start(out=ztab[16 * c : 16 * c + 1, :], in_=zrow)
            gbig = const.tile([P, S // 8], F32)
            nc.gpsimd.ap_gather(gbig[:], ztab[:], idx16[:], channels=P,
                                num_elems=S, d=1, num_idxs=S // 8)
            gv = gbig[:].rearrange("p (q f) -> p q f", q=16)
            nc.sync.dma_start(out=gath[:], in_=gv[::16, :, :])
        detect = const.tile([P, TS], F32)
        nc.vector.tensor_scalar(
            out=detect[:], in0=gath[:], scalar1=0.0, scalar2=None, op0=Alu.max
        )
        nc.sync.dma_start(out=detect_v, in_=detect[:])

        if variant == "nomid":
            nc.sync.dma_start(out=nt_v, in_=detect[:])
            scol = const.tile([P, TS], F32)
            nc.vector.tensor_copy(out=scol[:], in_=cnt_sb[:])
            for t in range(T - 1, -1, -1):
                if t in base_tiles:
                    bt = base_tiles[t]
                else:
                    bt = ld.tile([P, H], F32, tag="ld")
                    nc.sync.dma_start(out=bt[:], in_=base_v[:, t, :])
                nc.vector.scalar_tensor_tensor(
                    out=bt[:], in0=w_b[:], scalar=scol[:, t : t + 1], in1=bt[:],
                    op0=Alu.mult, op1=Alu.add,
                )
                nc.sync.dma_start(out=mixed_v[:, t, :], in_=bt[:])
            return

        # ---------------- tau = exact K-th largest of detect ----------------
        kth = const.tile([1, 2], F32)
        q = 1.0 - (K - 1.5) / (S - 1)  # k_adj = K-2  ->  out[0,1] = desc[K-1]
        nc.gpsimd.kth_largest(kth[:], detect[:], n_per_lane=TS, k=K, quantile=q)
        taub = const.tile([P, 2], F32)
        nc.gpsimd.partition_broadcast(taub[:], kth[:])
        tau = taub[:, 1:2]

        # ---------------- masks, tie-break, mean(topk) ----------------
        cs2 = const.tile([P, 2], F32)
        gt01 = const.tile([P, TS], F32)
        nc.vector.tensor_scalar(
            out=gt01[:], in0=detect[:], scalar1=tau, scalar2=None, op0=Alu.is_gt,
            op1=Alu.add, accum_out=cs2[:, 0:1],
        )
        selb = const.tile([P, TS], F32)
        nc.vector.scalar_tensor_tensor(
            out=selb[:], in0=detect[:], scalar=tau, in1=detect[:],
            op0=Alu.is_gt, op1=Alu.mult, accum_out=cs2[:, 1:2],
        )
        eq01 = const.tile([P, TS], F32)
        eqc = const.tile([P, 1], F32)
        nc.vector.tensor_scalar(
            out=eq01[:], in0=detect[:], scalar1=tau, scalar2=None, op0=Alu.is_equal,
            op1=Alu.add, accum_out=eqc[:],
        )
        bro = ps.tile([P, 2], F32)  # [:,0] = total #gt, [:,1] = total sum(gt*detect)
        nc.tensor.matmul(out=bro[:], lhsT=ones[:], rhs=cs2[:], start=True, stop=True)
        off = ps.tile([P, 1], F32)  # exclusive per-partition prefix of eq counts
        nc.tensor.matmul(out=off[:], lhsT=lt[:], rhs=eqc[:], start=True, stop=True)

        pcum = const.tile([P, TS], F32)
        nc.vector.tensor_tensor_scan(
            out=pcum[:], data0=eq01[:], data1=zer[:], initial=0.0,
            op0=Alu.add, op1=Alu.add,
        )
        rank = const.tile([P, TS], F32)
        nc.vector.tensor_scalar(
            out=rank[:], in0=pcum[:], scalar1=off[:], scalar2=None, op0=Alu.add
        )
        jb = const.tile([P, 1], F32)  # j = K - #gt
        nc.vector.tensor_scalar(
            out=jb[:], in0=bro[:, 0:1], scalar1=-1.0, scalar2=float(K),
            op0=Alu.mult, op1=Alu.add,
        )
        lem = const.tile([P, TS], F32)
        nc.vector.tensor_scalar(
            out=lem[:], in0=rank[:], scalar1=jb[:], scalar2=None, op0=Alu.is_le
        )
        zap = const.tile([P, TS], F32)
        nc.vector.tensor_tensor(out=zap[:], in0=lem[:], in1=eq01[:], op=Alu.mult)
        nc.vector.tensor_tensor(out=zap[:], in0=zap[:], in1=gt01[:], op=Alu.add)
        t4 = const.tile([P, TS], F32)
        nc.vector.tensor_tensor(out=t4[:], in0=detect[:], in1=zap[:], op=Alu.mult)
        nt = const.tile([P, TS], F32)
        nc.vector.tensor_tensor(out=nt[:], in0=detect[:], in1=t4[:], op=Alu.subtract)
        nc.sync.dma_start(out=nt_v, in_=nt[:])

        if dbg is not None:
            nc.sync.dma_start(
                out=dbg["z"].rearrange("(p t) -> p t", t=TS), in_=z_sb[:]
            )
            nc.sync.dma_start(
                out=dbg["gath"].rearrange("(p t) -> p t", t=TS), in_=gath[:]
            )
            nc.sync.dma_start(
                out=dbg["cnt"].rearrange("(p t) -> p t", t=TS), in_=cnt_sb[:]
            )
            nc.sync.dma_start(out=dbg["kth"], in_=kth[:])
            nc.sync.dma_start(
                out=dbg["rank"].rearrange("(p t) -> p t", t=TS), in_=rank[:]
            )

        # mean(topk) = (sum(detect>tau) + j*tau) / K, then fold in count
        t2 = const.tile([P, 1], F32)
        nc.vector.tensor_tensor(out=t2[:], in0=jb[:], in1=tau, op=Alu.mult)
        nc.vector.tensor_tensor(out=t2[:], in0=t2[:], in1=bro[:, 1:2], op=Alu.add)
        meanms = const.tile([P, 1], F32)
        nc.vector.tensor_scalar(
            out=meanms[:], in0=t2[:], scalar1=1.0 / K, scalar2=None, op0=Alu.mult
        )
        scol = const.tile([P, TS], F32)  # scol[p, t] = count * mean
        nc.vector.tensor_scalar(
            out=scol[:], in0=cnt_sb[:], scalar1=meanms[:], scalar2=None, op0=Alu.mult
        )

        # ---------------- pass 2: mixed = base + scol x w ----------------
        for t in range(T):  # non-resident first: loads prefetch in the middle
            if t in base_tiles:
                bt = base_tiles[t]
            else:
                bt = ld.tile([P, H], F32, tag="ld")
                nc.sync.dma_start(out=bt[:], in_=base_v[:, t, :])
            nc.vector.scalar_tensor_tensor(
                out=bt[:], in0=w_b[:], scalar=scol[:, t : t + 1], in1=bt[:],
                op0=Alu.mult, op1=Alu.add,
            )
            nc.sync.dma_start(out=mixed_v[:, t, :], in_=bt[:])


_NC_CACHE = None


def _build():
    global _NC_CACHE
    if _NC_CACHE is None:
        nc = bacc.Bacc("TRN2", target_bir_lowering=False, debug=False)
        base_t = nc.dram_tensor("base", [S, H], F32, kind="ExternalInput")
        pos_t = nc.dram_tensor("pos", [S], I32, kind="ExternalInput")
        w_t = nc.dram_tensor("w", [1, H], F32, kind="ExternalInput")
        mix_t = nc.dram_tensor("mixed", [S, H], F32, kind="ExternalOutput")
        det_t = nc.dram_tensor("detect", [S], F32, kind="ExternalOutput")
        nt_t = nc.dram_tensor("non_topk", [S], F32, kind="ExternalOutput")
        with TileContext(nc) as tc:
            emit(
                tc,
                (mix_t[:], det_t[:], nt_t[:]),
                (base_t[:], pos_t[:], w_t[:]),
            )
        nc.compile()
        _NC_CACHE = nc
    return _NC_CACHE


def run(base, intervention_positions, batch_weights, **spmd_kwargs):
    """Run the kernel; returns (outputs_tuple, BassKernelResults)."""
    nc = _build()
    base = np.asarray(base, dtype=np.float32)
    pos = np.asarray(intervention_positions, dtype=np.int32)
    w = np.asarray(batch_weights, dtype=np.float32)
    in_maps = [
        {
            "base": np.ascontiguousarray(base[i]),
            "pos": np.ascontiguousarray(pos[i]),
            "w": np.ascontiguousarray(w[i]),
        }
        for i in range(N_CORES)
    ]
    results = run_bass_kernel_spmd(nc, in_maps, list(range(N_CORES)), **spmd_kwargs)
    outs = results.results
    mixed = np.stack([outs[i]["mixed"] for i in range(N_CORES)])
    detect = np.stack([outs[i]["detect"] for i in range(N_CORES)])
    non_topk = np.stack([outs[i]["non_topk"] for i in range(N_CORES)])
    return (mixed, detect, non_topk), results


def kernel(base, intervention_positions, batch_weights):
    out, _ = run(base, intervention_positions, batch_weights)
    return out
